# revision 1
# baseline (speedup 1.0000x reference)
"""Level-1 3D Haar DWT on video [4,3,16,256,256] f32 -> 8 subbands
[4,3,8,128,128], pywt convention (cA=(x0+x1)/sqrt2, cD=(x0-x1)/sqrt2 over
frames, height, width).

Distribution: pure data parallel over the 8 frame pairs (F=16 -> 8
independent pairs); core k processes video[:, :, 2k:2k+2] with zero
cross-core communication.

Per-core pipeline (Bass/Tile), ragged chunks of CH pairs, row-half
u in {0,1} (rows u*128..u*128+127 of each frame):
  load (sync HWDGE ring): X[f,u] = x[p, f, u-half]  [128 rows, CH, 256]
  F stage (DVE):  A_u = X[0,u] + X[1,u]; D_u = X[0,u] - X[1,u]
  H stage (PE):   P_t_u = B.T @ (A|D)_u -> PSUM, B (+-1, fp32-exact):
                  out[j] = in[2j] + in[2j+1]      (aa rows 0..63)
                  out[64+j] = in[2j] - in[2j+1]   (ad rows 64..127)
                  TensorE has its own SBUF ports, so this runs fully
                  parallel to the DVE (GpSimd would lock the DVE's
                  second read port instead - measured, not theoretical).
  evac (ACT):     odd columns of P -> SBUF (a 2-input DVE op may read
                  at most one operand from PSUM)
  W stage (DVE):  (xe -/+ xo) * 2^-1.5 via the fused LN_BWD_DX custom
                  op; xe strided from PSUM, xo from SBUF; all three
                  1/sqrt2 stage scales folded here.
  store (scalar HWDGE ring): h-major DRAM layout, 512B+ runs.

Output DRAM y[u, j, t, e, p, w]: subband s = (t, j>=64, e), h = u*64+j%64.
"""

import math

import numpy as np

import concourse.bacc as bacc
import concourse.mybir as mybir
from concourse.bass_utils import run_bass_kernel_spmd
from concourse.tile import TileContext

F32 = mybir.dt.float32
NCORES = 8
NPAIRS = 12
CHUNKS = (2, 4, 4, 2)   # ragged: short first/last chunks trim fill/drain
CHMAX = max(CHUNKS)
NCHUNK = len(CHUNKS)
C3 = (1.0 / math.sqrt(2.0)) ** 3

_CACHE = {}


def _bmat():
    b = np.zeros((128, 128), np.float32)
    for j in range(64):
        b[2 * j, j] = 1.0
        b[2 * j + 1, j] = 1.0
        b[2 * j, 64 + j] = 1.0
        b[2 * j + 1, 64 + j] = -1.0
    return b


def _build_bass():
    nc = bacc.Bacc()
    x = nc.dram_tensor("x", [NPAIRS, 2, 256, 256], F32, kind="ExternalInput")
    bm = nc.dram_tensor("bmat", [128, 128], F32, kind="ExternalInput")
    y = nc.dram_tensor("y", [2, 128, 2, 2, NPAIRS, 128], F32,
                       kind="ExternalOutput")

    add = mybir.AluOpType.add
    sub = mybir.AluOpType.subtract

    with TileContext(nc) as tc:
        with tc.tile_pool(name="const", bufs=1) as cpool, \
             tc.tile_pool(name="io", bufs=3) as io_pool, \
             tc.tile_pool(name="mid", bufs=3) as mid_pool, \
             tc.tile_pool(name="ps", bufs=1, space="PSUM") as ps_pool:
            B = cpool.tile([128, 128], F32, name="B")
            nc.scalar.dma_start(out=B[:, :], in_=bm[:, :])
            p0 = 0
            for ci, CH in enumerate(CHUNKS):
                X = {}
                for u in range(2):
                    for f in range(2):
                        Xt = io_pool.tile([128, CH, 256], F32, name="X",
                                          tag=f"X{f}{u}", bufs=4,
                                          padded_shape=[128, CHMAX, 256])
                        nc.sync.dma_start(
                            out=Xt[:, :, :],
                            in_=x[p0:p0 + CH, f, 128 * u:128 * (u + 1)]
                                .rearrange("p r w -> r p w"),
                        )
                        X[(f, u)] = Xt
                # F stage: A_u = f0 + f1, D_u = f0 - f1
                AD = {}
                for u in range(2):
                    for t in range(2):       # 0: A (sum), 1: D (diff)
                        M = mid_pool.tile([128, CH, 256], F32, name="M",
                                          tag=f"M{t}{u}",
                                          padded_shape=[128, CHMAX, 256])
                        nc.vector.tensor_tensor(
                            M[:, :, :], X[(0, u)][:, :, :], X[(1, u)][:, :, :],
                            add if t == 0 else sub)
                        AD[(t, u)] = M
                # H stage on PE -> PSUM
                E = {}
                for (t, u), M in AD.items():
                    i = 2 * t + u
                    P = ps_pool.tile([128, CH, 256], F32, name="P", tag=f"P{i}",
                                     padded_shape=[128, CHMAX, 256])
                    Pf = P.rearrange("j p w -> j (p w)")
                    Mf = M.rearrange("j p w -> j (p w)")
                    for n0 in range(0, CH * 256, 512):  # one PSUM bank per matmul
                        n1 = min(n0 + 512, CH * 256)
                        nc.tensor.matmul(
                            Pf[:, n0:n1], B[:, :], Mf[:, n0:n1])
                    # evacuate only the odd columns (ACT): the W-stage
                    # 2-input op may read at most one operand from PSUM
                    Od = mid_pool.tile([128, CH * 128], F32, name="Od",
                                       tag=f"O{i}",
                                       padded_shape=[128, CHMAX * 128])
                    nc.scalar.copy(
                        Od[:, :],
                        P.rearrange("j p (w r) -> j (p w) r", r=2)[:, :, 1])
                    E[(t, u)] = (P, Od)
                # W stage (DVE): even cols from PSUM, odd from SBUF, *C3 fused
                for u in range(2):
                    YU = io_pool.tile([128, 2, 2, CH * 128], F32, name="YU",
                                      tag=f"YU{u}",
                                      padded_shape=[128, 2, 2, CHMAX * 128])
                    for t in range(2):
                        P, Od = E[(t, u)]
                        xe = P.rearrange("j p (w r) -> j (p w) r", r=2)[:, :, 0]
                        xo = Od[:, :]
                        # out = (in0 - in1*s0 - s1) * imm2
                        nc.vector.ln_bwd_dx(YU[:, t, 0, :], xe, xo, -1.0, 0.0, C3)
                        nc.vector.ln_bwd_dx(YU[:, t, 1, :], xe, xo, 1.0, 0.0, C3)
                    nc.scalar.dma_start(
                        out=y[u, :, :, :, p0:p0 + CH]
                            .rearrange("j t e p w -> j t e (p w)"),
                        in_=YU[:, :, :, :],
                    )
                p0 += CH
    nc.compile()
    return nc


def _get_nc():
    if "nc" not in _CACHE:
        _CACHE["nc"] = _build_bass()
    return _CACHE["nc"]


def _shard_inputs(video):
    video = np.ascontiguousarray(np.asarray(video), dtype=np.float32)
    bm = _bmat()
    in_maps = []
    for k in range(NCORES):
        shard = np.ascontiguousarray(
            video[:, :, 2 * k:2 * k + 2]).reshape(NPAIRS, 2, 256, 256)
        in_maps.append({"x": shard, "bmat": bm})
    return in_maps


def _unshard_outputs(results):
    # y[u, j, t, e, p, w]; j = qq*64 + jj; h = u*64 + jj; s = (t, qq, e)
    ys = np.stack([np.asarray(r["y"]) for r in results])  # [8,2,128,2,2,12,128]
    ys = ys.reshape(NCORES, 2, 2, 64, 2, 2, NPAIRS, 128)
    #      dims: (f, u, qq, jj, t, e, p, w)
    ys = ys.transpose(4, 2, 5, 6, 0, 1, 3, 7)
    #      -> (t, qq, e, p, f, u, jj, w)
    ys = ys.reshape(8, 4, 3, NCORES, 128, 128)            # (s, b, c, f, h, w)
    return tuple(np.ascontiguousarray(ys[s]) for s in range(8))


def run(video, **spmd_kwargs):
    nc = _get_nc()
    res = run_bass_kernel_spmd(
        nc, _shard_inputs(video), core_ids=list(range(NCORES)), **spmd_kwargs
    )
    return _unshard_outputs(res.results), res


def kernel(video):
    out, _ = run(video)
    return out



# revision 4
# speedup vs baseline: 1.1784x; 1.1784x over previous
"""Level-1 3D Haar DWT on video [4,3,16,256,256] f32 -> 8 subbands
[4,3,8,128,128], pywt convention (cA=(x0+x1)/sqrt2, cD=(x0-x1)/sqrt2 over
frames, height, width).

Distribution: pure data parallel over the 8 frame pairs (F=16 -> 8
independent pairs); core k processes video[:, :, 2k:2k+2] with zero
cross-core communication.

Per-core pipeline (Bass/Tile). HBM floor is ~35.2us (6.29MB in + 6.29MB
out at ~358 GB/s per core), so the design keeps every non-DMA engine
well under that and lets DMA run dense:

  load (sync HWDGE):  the whole 6.29MB input is resident in SBUF; all 12
                      pair-loads issue up front with no dependencies.
                      X[pair] = [128 part=(f,ro), 4 k, 256 w] where
                      partition (f*64+ro) holds input row 64k+ro of
                      frame f. 1KB descriptors (DRAM rows).
  F+H stage (PE):     ONE matmul per 512 cols with a +-1 stationary
                      matrix B2 combines frames (F) and row pairs (H):
                      P[g*32+jj, (k,w)] = st(t,f)*sq(q,o) summed over
                      partition (f, 2jj+o), g=2t+q. fp32-exact signs;
                      removes the old DVE F-stage entirely.
  evac (ACT):         odd w columns of P -> SBUF (a 2-input DVE op may
                      read at most one operand from PSUM).
  W stage (DVE):      (xe -/+ xo) * 2^-1.5 via fused LN_BWD_DX; xe
                      strided from PSUM, xo contiguous from SBUF; all
                      three 1/sqrt2 stage scales folded here.
  store (sync HWDGE): issued after the loads on the same ring; 4KB
                      contiguous runs per partition.

Output DRAM y[p', pair, e, k, ww]: p' = (2t+q)*32+jj, subband
s = (t, q, e), h = 32k+jj, w = ww.
"""

import math

import numpy as np

import concourse.bacc as bacc
import concourse.mybir as mybir
from concourse.bass_utils import run_bass_kernel_spmd
from concourse.tile import TileContext

F32 = mybir.dt.float32
NCORES = 8
NPAIRS = 12
C3 = (1.0 / math.sqrt(2.0)) ** 3

_CACHE = {}


def _bmat():
    # B2[f*64 + 2*jj + o, (2t+q)*32 + jj] = st * sq
    # st: frame sign (t=0: ++, t=1: +-), sq: row-in-pair sign (q=0: ++, q=1: +-)
    b = np.zeros((128, 128), np.float32)
    for t in range(2):
        for q in range(2):
            g = 2 * t + q
            for f in range(2):
                st = 1.0 if (t == 0 or f == 0) else -1.0
                for o in range(2):
                    sq = 1.0 if (q == 0 or o == 0) else -1.0
                    for jj in range(32):
                        b[f * 64 + 2 * jj + o, g * 32 + jj] = st * sq
    return b


def _build_bass():
    nc = bacc.Bacc()
    x = nc.dram_tensor("x", [NPAIRS, 2, 256, 256], F32, kind="ExternalInput")
    bm = nc.dram_tensor("bmat", [128, 128], F32, kind="ExternalInput")
    y = nc.dram_tensor("y", [128, NPAIRS, 1024], F32, kind="ExternalOutput")

    with TileContext(nc) as tc:
        with tc.tile_pool(name="const", bufs=1) as cpool, \
             tc.tile_pool(name="xin", bufs=NPAIRS) as x_pool, \
             tc.tile_pool(name="mid", bufs=3) as mid_pool, \
             tc.tile_pool(name="out", bufs=4) as out_pool, \
             tc.tile_pool(name="ps", bufs=4, space="PSUM") as ps_pool:
            B = cpool.tile([128, 128], F32, name="B")
            nc.scalar.dma_start(out=B[:, :], in_=bm[:, :])

            # All input loads issue up front; the whole input is SBUF-resident.
            X = []
            for p in range(NPAIRS):
                Xt = x_pool.tile([128, 4, 256], F32, name="X", tag="X")
                for f in range(2):
                    nc.sync.dma_start(
                        out=Xt[64 * f:64 * (f + 1), :, :],
                        in_=x[p, f].rearrange("(k ro) w -> ro k w", k=4),
                    )
                X.append(Xt)

            for p in range(NPAIRS):
                Mf = X[p].rearrange("j k w -> j (k w)")
                P = ps_pool.tile([128, 1024], F32, name="P", tag="P")
                for n0 in range(0, 1024, 512):  # one PSUM bank per matmul
                    nc.tensor.matmul(P[:, n0:n0 + 512], B[:, :],
                                     Mf[:, n0:n0 + 512])
                Pe = P.rearrange("j (n r) -> j n r", r=2)
                # evacuate odd columns (ACT): the W-stage 2-input DVE op
                # may read at most one operand from PSUM
                Od = mid_pool.tile([128, 512], F32, name="Od", tag="Od")
                nc.scalar.copy(Od[:, :], Pe[:, :, 1])
                # W stage (DVE): even cols from PSUM, odd from SBUF, *C3
                YU = out_pool.tile([128, 2, 512], F32, name="YU", tag="YU")
                xe = Pe[:, :, 0]
                nc.vector.ln_bwd_dx(YU[:, 0, :], xe, Od[:, :], -1.0, 0.0, C3)
                nc.vector.ln_bwd_dx(YU[:, 1, :], xe, Od[:, :], 1.0, 0.0, C3)
                nc.sync.dma_start(
                    out=y[:, p, :],
                    in_=YU.rearrange("j e n -> j (e n)"),
                )
    nc.compile()
    return nc


def _get_nc():
    if "nc" not in _CACHE:
        _CACHE["nc"] = _build_bass()
    return _CACHE["nc"]


def _shard_inputs(video):
    video = np.ascontiguousarray(np.asarray(video), dtype=np.float32)
    bm = _bmat()
    in_maps = []
    for k in range(NCORES):
        shard = np.ascontiguousarray(
            video[:, :, 2 * k:2 * k + 2]).reshape(NPAIRS, 2, 256, 256)
        in_maps.append({"x": shard, "bmat": bm})
    return in_maps


def _unshard_outputs(results):
    # y[p', pair, n]: p' = (2t+q)*32 + jj, n = e*512 + k*128 + ww
    ys = np.stack([np.asarray(r["y"]) for r in results])  # [8,128,12,1024]
    ys = ys.reshape(NCORES, 2, 2, 32, 4, 3, 2, 4, 128)
    #      dims: (core, t, q, jj, b, c, e, k, ww)
    ys = ys.transpose(1, 2, 6, 4, 5, 0, 7, 3, 8)
    #      -> (t, q, e, b, c, core, k, jj, ww)
    ys = ys.reshape(8, 4, 3, NCORES, 128, 128)            # (s, b, c, f, h, w)
    return tuple(np.ascontiguousarray(ys[s]) for s in range(8))


def run(video, **spmd_kwargs):
    nc = _get_nc()
    res = run_bass_kernel_spmd(
        nc, _shard_inputs(video), core_ids=list(range(NCORES)), **spmd_kwargs
    )
    return _unshard_outputs(res.results), res


def kernel(video):
    out, _ = run(video)
    return out
